# revision 16
# baseline (speedup 1.0000x reference)
"""Trainium2 Bass kernel for nn_AvgPoolingModel (embedding avg-pool + tiny MLP).

Model:  emb = table[batch]           # [B, L, 300] gather
        pooled = emb.sum(1) / lens   # [B, 300]
        h1 = relu(pooled @ W1.T + b1)
        h2 = relu(h1 @ W2.T + b2)
        y  = (h2 @ W3.T + b3)[:, 0]  # [B]

Sharding: data-parallel over B across 8 cores (512 rows/core); embedding
table + MLP weights replicated per core.

Per-core kernel: 4 row-tiles of 128 rows.  HW indirect DMA consumes one
index per partition per call, so each call gathers table rows for one
sequence position of 128 batch rows ([128, 300], 128 descriptors).  Calls
land in a [128, RING*300] ring tile (double-buffered); a DVE strided
reduce collapses each full ring into the row-tile accumulator, overlapped
under the Pool-engine DMA issue stream (the bottleneck at ~1.2-1.6us per
gather call).  The 3-layer MLP runs on the PE with on-chip transposes;
biases enter via rank-1 ones-vector matmuls into the same PSUM
accumulation group; 1/lens is precomputed host-side and applied on DVE.
"""

import numpy as np

import concourse.bass as bass
import concourse.mybir as mybir
from concourse.tile import TileContext

VOCAB, EMB = 100000, 300
B, L = 4096, 200
H1, H2 = 150, 150
NCORES = 8
BC = B // NCORES  # rows per core
P = 128
NT = BC // P      # row-tiles per core
RING = 10         # gather calls per ring fill
NCHUNK = L // RING

F32 = mybir.dt.float32
I32 = mybir.dt.int32


def build_nc(repeat=None, style="ring", single_packet=False):
    """Build the per-core Bass kernel.

    repeat=None: the real kernel.  repeat=R: the gather+reduce+MLP body is
    wrapped in a hardware For_i loop executing R times (identical work per
    iteration) — used only for wall-clock timing, where slope over R
    isolates HW exec time from the ~100ms axon dispatch overhead.

    style="ring": gathers land in [P, RING*EMB] ring tiles, DVE strided
    reduce per ring fill.  style="tiles": every gather gets its own
    [P, EMB] tile from a deep rotation (no same-tile WAW at all); DVE
    accumulates with one tensor_add per gather.
    """
    from concourse import bacc
    from concourse.masks import make_identity

    nc = bacc.Bacc("TRN2", target_bir_lowering=False, debug=False)

    batch_d = nc.dram_tensor("batch", [BC, L], I32, kind="ExternalInput")
    recip_d = nc.dram_tensor("recip", [BC], F32, kind="ExternalInput")
    emb_d = nc.dram_tensor("emb_table", [VOCAB, EMB], F32, kind="ExternalInput")
    w1t_d = nc.dram_tensor("w1t", [EMB, H1], F32, kind="ExternalInput")
    b1_d = nc.dram_tensor("b1", [H1], F32, kind="ExternalInput")
    w2t_d = nc.dram_tensor("w2t", [H1, H2], F32, kind="ExternalInput")
    b2_d = nc.dram_tensor("b2", [H2], F32, kind="ExternalInput")
    w3t_d = nc.dram_tensor("w3t", [H2, 1], F32, kind="ExternalInput")
    b3_d = nc.dram_tensor("b3", [1], F32, kind="ExternalInput")
    y_d = nc.dram_tensor("y", [BC], F32, kind="ExternalOutput")

    with TileContext(nc) as tc:
        with (
            tc.tile_pool(name="const", bufs=1) as cpool,
            tc.tile_pool(name="gat", bufs=1) as gpool,
            tc.tile_pool(name="ring", bufs=2) as rpool,
            tc.tile_pool(name="work", bufs=2) as wpool,
            tc.tile_pool(name="psum", bufs=1, space="PSUM") as ppool,
            tc.tile_pool(name="psum2", bufs=2, space="PSUM") as ppool2,
        ):
            # ---- one-time constants -------------------------------------
            identity = cpool.tile([P, P], F32)
            make_identity(nc, identity[:])
            ones_row = cpool.tile([1, P], F32)
            nc.vector.memset(ones_row[:], 1.0)

            w1t_sb = cpool.tile([100, 3 * H1], F32)   # 3 K-chunks of W1.T
            for c in range(3):
                nc.sync.dma_start(
                    out=w1t_sb[:, c * H1:(c + 1) * H1],
                    in_=w1t_d[c * 100:(c + 1) * 100, :],
                )
            w2t_sb = cpool.tile([75, 2 * H2], F32)    # 2 K-chunks of W2.T
            for c in range(2):
                nc.sync.dma_start(
                    out=w2t_sb[:, c * H2:(c + 1) * H2],
                    in_=w2t_d[c * 75:(c + 1) * 75, :],
                )
            w3t_sb = cpool.tile([75, 2], F32)         # 2 K-chunks of W3.T
            for c in range(2):
                nc.sync.dma_start(
                    out=w3t_sb[:, c:c + 1], in_=w3t_d[c * 75:(c + 1) * 75, :]
                )
            b1_sb = cpool.tile([1, H1], F32)
            nc.sync.dma_start(out=b1_sb[:], in_=b1_d[None, :])
            b2_sb = cpool.tile([1, H2], F32)
            nc.sync.dma_start(out=b2_sb[:], in_=b2_d[None, :])
            b3_sb = cpool.tile([1, 1], F32)
            nc.sync.dma_start(out=b3_sb[:], in_=b3_d[None, :])

            recip_sb = cpool.tile([P, NT], F32)
            nc.sync.dma_start(
                out=recip_sb[:], in_=recip_d.ap().rearrange("(t p) -> p t", p=P)
            )
            out_sb = cpool.tile([P, NT], F32)

            # ---- per-row-tile index tiles and accumulators --------------
            batch_sbs, accs = [], []
            for t in range(NT):
                batch_sb = gpool.tile([P, L], I32, tag=f"batch{t}", name=f"batch_sb{t}")
                nc.sync.dma_start(out=batch_sb[:], in_=batch_d[t * P:(t + 1) * P, :])
                batch_sbs.append(batch_sb)
                acc = gpool.tile([P, EMB], F32, tag=f"acc{t}", name=f"acc{t}")
                accs.append(acc)

            # ---- gather stream (Pool-bound) with overlapped DVE reduce --
            def gather_and_reduce(it=""):
                if style == "tiles":
                    gather_and_reduce_tiles(it)
                    return
                # Gathers are interleaved across the NT ring tiles so that
                # consecutive Pool DMAs never target the same tile: same-tile
                # writes carry a WAW completion dependency (~3us) which would
                # otherwise throttle the issue stream to completion latency.
                for c in range(NCHUNK):
                    rings = [
                        rpool.tile([P, RING * EMB], F32, tag=f"ring{t}",
                                   name=f"ring{t}_{c}{it}")
                        for t in range(NT)
                    ]
                    for k in range(RING):
                        for t in range(NT):
                            inst = nc.gpsimd.indirect_dma_start(
                                out=rings[t][:, k * EMB:(k + 1) * EMB],
                                out_offset=None,
                                in_=emb_d[:],
                                in_offset=bass.IndirectOffsetOnAxis(
                                    ap=batch_sbs[t][:, c * RING + k:c * RING + k + 1],
                                    axis=0,
                                ),
                            )
                            if single_packet:
                                inst.ins.single_packet = True
                    for t in range(NT):
                        rview = rings[t].rearrange("p (k e) -> p e k", e=EMB)
                        if c == 0:
                            nc.vector.reduce_sum(
                                accs[t][:], rview, axis=mybir.AxisListType.X
                            )
                        else:
                            part = wpool.tile([P, EMB], F32, tag=f"part{t}",
                                              name=f"part{t}_{c}{it}")
                            nc.vector.reduce_sum(
                                part[:], rview, axis=mybir.AxisListType.X
                            )
                            nc.vector.tensor_add(
                                out=accs[t][:], in0=accs[t][:], in1=part[:]
                            )

            def gather_and_reduce_tiles(it=""):
                # One private [P, EMB] tile per gather (deep rotation, no
                # same-tile WAW); DVE accumulates each into acc[t].  The
                # accumulation order interleaves t so DVE tracks ~NT gathers
                # behind the DMA stream.
                for l in range(L):
                    for t in range(NT):
                        g = rpool.tile([P, EMB], F32, tag="gt", bufs=32,
                                       name=f"g{t}_{l}{it}")
                        nc.gpsimd.indirect_dma_start(
                            out=g[:],
                            out_offset=None,
                            in_=emb_d[:],
                            in_offset=bass.IndirectOffsetOnAxis(
                                ap=batch_sbs[t][:, l:l + 1], axis=0
                            ),
                        )
                        if l == 0:
                            nc.vector.tensor_copy(out=accs[t][:], in_=g[:])
                        else:
                            nc.vector.tensor_add(
                                out=accs[t][:], in0=accs[t][:], in1=g[:]
                            )

            # ---- per-row-tile epilogue: scale + MLP ---------------------
            def epilogue(it=""):
                for t in range(NT):
                    acc = accs[t]
                    scaled = wpool.tile([P, EMB], F32, tag="scaled",
                                        name=f"scaled{t}{it}")
                    nc.vector.tensor_scalar_mul(
                        scaled[:], acc[:], recip_sb[:, t:t + 1]
                    )

                    pooledT = wpool.tile([100, 3 * P], F32, tag="pooledT",
                                         name=f"pooledT{t}{it}")
                    for c in range(3):
                        tp_ps = ppool2.tile([100, P], F32, tag="tps",
                                            name=f"tp{t}_{c}{it}")
                        nc.tensor.transpose(
                            out=tp_ps[:], in_=scaled[:, c * 100:(c + 1) * 100],
                            identity=identity[:],
                        )
                        nc.scalar.copy(pooledT[:, c * P:(c + 1) * P], tp_ps[:])

                    h1_ps = ppool.tile([P, H1], F32, tag="h1", name=f"h1ps{t}{it}")
                    for c in range(3):
                        nc.tensor.matmul(
                            out=h1_ps[:],
                            lhsT=pooledT[:, c * P:(c + 1) * P],
                            rhs=w1t_sb[:, c * H1:(c + 1) * H1],
                            start=(c == 0), stop=False,
                        )
                    nc.tensor.matmul(
                        out=h1_ps[:], lhsT=ones_row[:], rhs=b1_sb[:],
                        start=False, stop=True,
                    )
                    h1_sb = wpool.tile([P, H1], F32, tag="h1sb", name=f"h1sb{t}{it}")
                    nc.scalar.activation(
                        h1_sb[:], h1_ps[:], mybir.ActivationFunctionType.Relu
                    )

                    h1t = wpool.tile([75, 2 * P], F32, tag="h1t", name=f"h1t{t}{it}")
                    for c in range(2):
                        t1_ps = ppool2.tile([75, P], F32, tag="tps",
                                            name=f"t1{t}_{c}{it}")
                        nc.tensor.transpose(
                            out=t1_ps[:], in_=h1_sb[:, c * 75:(c + 1) * 75],
                            identity=identity[:],
                        )
                        nc.scalar.copy(h1t[:, c * P:(c + 1) * P], t1_ps[:])

                    h2_ps = ppool.tile([P, H2], F32, tag="h2", name=f"h2ps{t}{it}")
                    for c in range(2):
                        nc.tensor.matmul(
                            out=h2_ps[:],
                            lhsT=h1t[:, c * P:(c + 1) * P],
                            rhs=w2t_sb[:, c * H2:(c + 1) * H2],
                            start=(c == 0), stop=False,
                        )
                    nc.tensor.matmul(
                        out=h2_ps[:], lhsT=ones_row[:], rhs=b2_sb[:],
                        start=False, stop=True,
                    )
                    h2_sb = wpool.tile([P, H2], F32, tag="h2sb", name=f"h2sb{t}{it}")
                    nc.scalar.activation(
                        h2_sb[:], h2_ps[:], mybir.ActivationFunctionType.Relu
                    )

                    h2t = wpool.tile([75, 2 * P], F32, tag="h2t", name=f"h2t{t}{it}")
                    for c in range(2):
                        t2_ps = ppool2.tile([75, P], F32, tag="tps",
                                            name=f"t2{t}_{c}{it}")
                        nc.tensor.transpose(
                            out=t2_ps[:], in_=h2_sb[:, c * 75:(c + 1) * 75],
                            identity=identity[:],
                        )
                        nc.scalar.copy(h2t[:, c * P:(c + 1) * P], t2_ps[:])

                    y_ps = ppool.tile([P, 1], F32, tag="y", name=f"yps{t}{it}")
                    for c in range(2):
                        nc.tensor.matmul(
                            out=y_ps[:],
                            lhsT=h2t[:, c * P:(c + 1) * P],
                            rhs=w3t_sb[:, c:c + 1],
                            start=(c == 0), stop=False,
                        )
                    nc.tensor.matmul(
                        out=y_ps[:], lhsT=ones_row[:], rhs=b3_sb[:],
                        start=False, stop=True,
                    )
                    nc.scalar.copy(out_sb[:, t:t + 1], y_ps[:])

                nc.sync.dma_start(
                    out=y_d.ap().rearrange("(t p) -> p t", p=P), in_=out_sb[:]
                )

            if repeat is None:
                gather_and_reduce()
                epilogue()
            else:
                with tc.For_i(0, repeat, 1) as _i:
                    gather_and_reduce()
                    epilogue()

    nc.compile()
    return nc


def prep_in_maps(batch, lens, emb_table, W1, b1, W2, b2, W3, b3):
    batch = np.ascontiguousarray(np.asarray(batch, dtype=np.int32))
    lens_f = np.asarray(lens).astype(np.float32)
    recip = (np.float32(1.0) / lens_f).astype(np.float32)
    emb_table = np.ascontiguousarray(np.asarray(emb_table, dtype=np.float32))
    common = {
        "emb_table": emb_table,
        "w1t": np.ascontiguousarray(np.asarray(W1, np.float32).T),
        "b1": np.asarray(b1, np.float32),
        "w2t": np.ascontiguousarray(np.asarray(W2, np.float32).T),
        "b2": np.asarray(b2, np.float32),
        "w3t": np.ascontiguousarray(np.asarray(W3, np.float32).T),
        "b3": np.asarray(b3, np.float32),
    }
    in_maps = []
    for c in range(NCORES):
        sl = slice(c * BC, (c + 1) * BC)
        in_maps.append({"batch": batch[sl], "recip": recip[sl], **common})
    return in_maps


_NC_CACHE = {}


def kernel(batch, lens, emb_table, W1, b1, W2, b2, W3, b3):
    from concourse.bass_utils import run_bass_kernel_spmd

    if "nc" not in _NC_CACHE:
        _NC_CACHE["nc"] = build_nc()
    nc = _NC_CACHE["nc"]
    in_maps = prep_in_maps(batch, lens, emb_table, W1, b1, W2, b2, W3, b3)
    last_err = None
    for _attempt in range(3):
        try:
            res = run_bass_kernel_spmd(nc, in_maps, core_ids=list(range(NCORES)))
            break
        except Exception as e:  # transient axon desync/device-state errors
            last_err = e
            import time as _time

            _time.sleep(5.0)
    else:
        raise last_err
    out = np.concatenate([r["y"] for r in res.results])
    return out.astype(np.float32)
